# revision 23
# baseline (speedup 1.0000x reference)
"""Multi-head causal attention (B=2, T=2048, C=1024, H=16) on 8 Trainium2
NeuronCores, tensor-parallel over heads (2 heads per core).

v2: trace-driven rework of the v1 software-pipelined kernel.

  - V is produced token-major directly in phase-1 (lhsT = x token tiles,
    rhs = w_v columns): the 32 PE transposes, their identity loads and
    the vt/tp DVE staging copies are gone, along with the DVE-queue
    head-of-line tangle they caused at chunk boundaries (exp -> mask ->
    vt-copy -> transpose -> AV cycles cost ~4.6us stalls + HAM cold).
  - both heads' AV accumulators live in one [65, 2*TC] PSUM tile
    (adjacent banks): one DVE copy grabs both Z rows, reciprocal runs
    once on [1, 2*TC], and the PE bch matmuls broadcast the reciprocal
    (the per-head [64, TC] reciprocals are gone).
  - ScalarE does nothing but exp (it was 57% busy with copies): all
    PSUM evacuations (phase-1 bias adds, phase-4 out-proj copies, norm)
    run on DVE; V bias is folded into phase-1 as a rank-1 ones matmul.
  - chunk order ends with a 4-ktile chunk (b1 qc=0) so the big second-
    to-last chunk's out-projection overlaps that chunk's k-loop instead
    of draining after everything else; output DMA is per-m (128 KB).

Host-side layouts are pre-tiled for contiguous DMA rows; the 8 partial
[C, BT] outputs are summed (and bias added) on the host.
"""

import os
import sys
from collections import deque

for _p in ("/opt/trn_rl_repo", "/root/.axon_site/_ro/trn_rl_repo"):
    if os.path.isdir(_p) and _p not in sys.path:
        sys.path.insert(0, _p)

import ml_dtypes
import numpy as np

import concourse.bacc as bacc
import concourse.bass as bass
import concourse.mybir as mybir
import concourse.tile as tile
from concourse.bass_utils import run_bass_kernel_spmd

B, T, C, H, D = 2, 2048, 1024, 16, 64
NCORES = 8
BT = B * T                      # 4096 flattened tokens
TC = 512                        # token chunk (matmul free dim)
NTC = BT // TC                  # 8 token chunks
FP = mybir.dt.float32
FPR = mybir.dt.float32r
BF = mybir.dt.bfloat16
ACT = mybir.ActivationFunctionType
AV_DELAY = 3                    # k-tiles the AV matmul trails the scores

# chunk processing order: logical chunk ids (b*4 + qc).
CHUNK_ORDER = [0, 1, 2, 3, 4, 5, 6, 7]
# phase-1 filler emitted during each position (chunk ids); chunk 0's
# phase-1 runs inline before the loop.
P1_AT_POS = {0: [1], 1: [2], 2: [3], 3: [4], 4: [5], 5: [6], 6: [7], 7: []}

LAST_RESULTS = None             # stashed BassKernelResults for test harness


def build_nc():
    nc = bacc.Bacc(None, target_bir_lowering=False, debug=False)

    xt = nc.declare_dram_parameter("xt", [C, BT], BF, isOutput=False)
    wc = nc.declare_dram_parameter("wc", [128, 3072], BF, isOutput=False)
    wout = nc.declare_dram_parameter("wout", [128, C], BF, isOutput=False)
    bqkv = nc.declare_dram_parameter("bqkv", [128, 3], FP, isOutput=False)
    bvrow = nc.declare_dram_parameter("bvrow", [1, 128], BF, isOutput=False)
    masks = nc.declare_dram_parameter("masks", [128, 256], BF, isOutput=False)
    onesr = nc.declare_dram_parameter("onesr", [1, 64], FP, isOutput=False)
    onesb = nc.declare_dram_parameter("onesb", [1, 128], BF, isOutput=False)
    # yt[i*128 + p, m*512 + t] = y_partial[m*128 + p, i*512 + t]
    yt = nc.declare_dram_parameter("yt", [C, BT], BF, isOutput=True)

    with tile.TileContext(nc) as tc:
        with (
            tc.tile_pool(name="const", bufs=1) as cpool,
            tc.tile_pool(name="big", bufs=1) as bigpool,
            tc.tile_pool(name="sb", bufs=2) as sbpool,
            tc.tile_pool(name="ps", bufs=2, space="PSUM") as pspool,
        ):
            # ---- constants; DMA order = startup order ----
            # wc split per k-slice so the first phase-1 matmul only waits
            # on a 96 KB transfer; chunk-0 x columns land first. DMAs are
            # spread over sync+gpsimd dynamic queues ONLY: a trigger on
            # the ScalarE queue blocks exp behind DMA-completion waits
            # (measured 5 us PE stalls in the first third of the kernel).
            dma_engs = (nc.sync, nc.gpsimd)
            ndma = 0

            def dma(out, in_):
                nonlocal ndma
                dma_engs[ndma % len(dma_engs)].dma_start(out=out, in_=in_)
                ndma += 1

            wc_sb = cpool.tile([128, 3072], BF)
            xks = []
            for k in range(8):
                xk = cpool.tile([128, BT], BF, name=f"xk{k}")
                xks.append(xk)
            for k in range(8):
                dma(wc_sb[:, k * 384:(k + 1) * 384],
                    wc[:, k * 384:(k + 1) * 384])
                dma(xks[k][:, 0:TC], xt[k * 128:(k + 1) * 128, 0:TC])
            # small constants BEFORE the bulk x stream: chunk 0's mask
            # multiply head-blocked the DVE FIFO ~7 us waiting for a
            # masks tile queued behind 7 MB of x columns.
            bq_sb = cpool.tile([128, 3], FP)
            dma(bq_sb[:], bqkv[:, :])
            bv_sb = cpool.tile([1, 128], BF)
            dma(bv_sb[:], bvrow[:, :])
            onesb_sb = cpool.tile([1, 128], BF)
            dma(onesb_sb[:], onesb[:, :])
            masks_sb = cpool.tile([128, 256], BF)
            dma(masks_sb[:], masks[:, :])
            onesr_sb = cpool.tile([1, 64], FPR)
            dma(onesr_sb[:], onesr.bitcast(FPR)[:, :])
            wout_sb = cpool.tile([128, C], BF)
            dma(wout_sb[:], wout[:, :])
            # remaining x columns, chunk-pair major so chunk c's slice
            # lands before phase-1(c) needs it
            for c0 in range(TC, BT, 2 * TC):
                c1 = min(c0 + 2 * TC, BT)
                for k in range(8):
                    dma(xks[k][:, c0:c1], xt[k * 128:(k + 1) * 128, c0:c1])

            # ---- HAM warm-up: keep the PE busy on scratch data while the
            # initial DMAs land, so real work starts at 2.4 GHz ----
            dummy = cpool.tile([128, 128], BF, name="dummy")
            nc.vector.memset(dummy[:], 0.0)
            for _ in range(56):
                dps = pspool.tile([128, 128], FP, tag="m", bufs=2,
                                  name="dps")
                nc.tensor.matmul(dps[:], dummy[:], dummy[:],
                                 start=True, stop=True)

            # ---- persistent intermediates ----
            QT = bigpool.tile([128, BT], BF)
            KT = bigpool.tile([128, BT], BF)
            # V in [token, dim] layout, 130 cols per 128-token block:
            # [V_h0 (64) | ones | V_h1 (64) | ones]; memset once puts the
            # ones columns in place, the phase-1 V copy overwrites the rest.
            vaug = bigpool.tile([128, 32 * 130], BF)
            nc.vector.memset(vaug[:], 1.0)

            p1state = {}

            def make_phase1_units(i):
                """QKV projection for chunk i in ~2-matmul granules.
                Q/K land dim-major in QT/KT; V lands token-major in a
                PSUM tile (lhsT = x token tiles) and is copied into vaug
                with one DVE op — no PE transposes."""
                t0 = i * TC

                def qk_granule(g, k0):
                    def emit():
                        if k0 == 0:
                            p1state[(i, g)] = pspool.tile(
                                [128, TC], FP, tag="m", bufs=2,
                                name=f"p1q{i}g{g}")
                        ps = p1state[(i, g)]
                        for k in (k0, k0 + 1):
                            nc.tensor.matmul(
                                ps[:],
                                wc_sb[:, k * 384 + g * 128:
                                      k * 384 + (g + 1) * 128],
                                xks[k][:, t0:t0 + TC],
                                start=(k == 0), stop=(k == 7),
                            )
                        if k0 == 6:
                            del p1state[(i, g)]
                            nc.vector.tensor_scalar_add(
                                out=(QT, KT)[g][:, t0:t0 + TC], in0=ps[:],
                                scalar1=bq_sb[:, g:g + 1])
                    return emit

                def v_granule(j):
                    # one token-tile per unit (8 matmuls + bias): long
                    # back-to-back PE runs keep the LDW pipeline primed
                    def emit():
                        if j == 0:
                            p1state[(i, 'v')] = pspool.tile(
                                [128, TC], FP, tag="m", bufs=2,
                                name=f"p1v{i}")
                        ps = p1state[(i, 'v')]
                        for k in range(8):
                            nc.tensor.matmul(
                                ps[:, j * 128:(j + 1) * 128],
                                xks[k][:, t0 + j * 128:t0 + (j + 1) * 128],
                                wc_sb[:, k * 384 + 256:k * 384 + 384],
                                start=(k == 0), stop=False,
                            )
                        # fold the V bias in as a rank-1 ones matmul
                        nc.tensor.matmul(
                            ps[:, j * 128:(j + 1) * 128],
                            onesb_sb[0:1, :], bv_sb[0:1, :],
                            start=False, stop=True,
                        )
                    return emit

                def v_copy():
                    ps = p1state.pop((i, 'v'))
                    nc.vector.tensor_copy(
                        vaug[:].rearrange(
                            "p (j a c) -> p j a c", a=2, c=65)[
                            :, i * 4:(i + 1) * 4, :, 0:64],
                        ps[:].rearrange("p (j a c) -> p j a c", j=4, c=64),
                    )

                qk = [((i, 'qk'), qk_granule(g, k0)) for g in range(2)
                      for k0 in range(0, 8, 2)]
                v = [((i, 'v'), v_granule(j)) for j in range(4)]
                v.append(((i, 'v'), v_copy))
                return qk + v

            filler = deque()

            def pop_fillers(k):
                for _ in range(k):
                    if filler:
                        filler.popleft()[1]()

            def drain_through(key):
                """Pop until no unit with the given key remains (they form a
                contiguous run in FIFO order)."""
                while any(u[0] == key for u in filler):
                    filler.popleft()[1]()

            # state deferred from the previous chunk
            pending = None  # (i, b, n, otp, pts, emit_av)

            def finish_pending():
                """Tail AVs for the previous chunk (emitted directly);
                normalization + phase-4 are returned as filler units."""
                nonlocal pending
                if pending is None:
                    return []
                pi, pb, pn, potp, ppts, pemit_av = pending
                pending = None
                for j in range(max(pn - AV_DELAY, 0), pn):
                    pemit_av(j)
                state = {}

                def norm_unit():
                    # both heads' Z rows sit at partition 64 of the two
                    # adjacent banks of potp: one f32r copy, PE broadcast
                    # (bch), then reciprocal + multiply per head on DVE.
                    rz = sbpool.tile([1, 2 * TC], FPR, tag="rz", bufs=2,
                                     name=f"rz{pi}")
                    with nc.allow_low_precision(reason="sums f32r"):
                        nc.scalar.copy(rz[:], potp[64:65, :])
                    ot = sbpool.tile([128, TC], BF, tag="ot", bufs=2,
                                     name=f"ot{pi}")
                    for h in range(2):
                        bchh = pspool.tile([64, TC], FP, tag="m", bufs=2,
                                           name=f"bch{h}_{pi}")
                        nc.tensor.matmul(bchh[:], onesr_sb[0:1, :],
                                         rz[0:1, h * TC:(h + 1) * TC],
                                         start=True, stop=True)
                        bcsh = sbpool.tile([64, TC], FP, tag=f"bcs{h}",
                                           bufs=2, name=f"bcs{h}_{pi}")
                        nc.vector.reciprocal_approx_fast(out=bcsh[:],
                                                         in_=bchh[:])
                        nc.vector.tensor_mul(
                            ot[h * 64:(h + 1) * 64, :],
                            potp[0:64, h * TC:(h + 1) * TC], bcsh[:])
                    state['ot'] = ot
                    state['ys'] = sbpool.tile([128, BT], BF, tag="ys",
                                              bufs=2, name=f"ys{pi}")

                def m_unit(m, last):
                    # PSUM evacuation engine: mostly DVE; ScalarE takes a
                    # couple per chunk (it has slack between exps), and on
                    # the final chunk (no exp left) strict alternation
                    # halves the copy-bound tail chain.
                    if last:
                        ceng = (nc.vector.tensor_copy,
                                nc.scalar.copy)[m % 2]
                    else:
                        ceng = (nc.vector.tensor_copy if m % 4 != 3
                                else nc.scalar.copy)

                    def emit():
                        yp = pspool.tile([128, TC], FP, tag="m", bufs=2,
                                         name=f"yp{pi}_{m}")
                        nc.tensor.matmul(
                            yp[:], wout_sb[:, m * 128:(m + 1) * 128],
                            state['ot'][:], start=True, stop=True)
                        ys = state['ys']
                        ceng(ys[:, m * TC:(m + 1) * TC], yp[:])
                        if m == 7:
                            # the chunk's rows of yt form one contiguous
                            # 1 MB DRAM region: two half-partition DMAs
                            # (contiguous 8 KB bursts) instead of per-m
                            # column slices (1 KB strided bursts, which
                            # were packet-overhead-bound at the tail)
                            for q, eng in enumerate((nc.sync, nc.gpsimd)):
                                eng.dma_start(
                                    out=yt[pi * 128 + q * 64:
                                           pi * 128 + (q + 1) * 64, :],
                                    in_=ys[q * 64:(q + 1) * 64, :])
                    return emit

                last = pi == CHUNK_ORDER[-1]
                return [((pi, 'p4'), norm_unit)] + \
                    [((pi, 'p4'), m_unit(m, last)) for m in range(8)]

            for pos, tcx in enumerate(CHUNK_ORDER):
                b, qc = divmod(tcx, 4)
                t0 = tcx * TC
                n = 4 * (qc + 1)

                if pos == 0:
                    for _, u in make_phase1_units(tcx):
                        u()
                    for nxt in P1_AT_POS[pos]:
                        filler.extend(make_phase1_units(nxt))
                else:
                    # phase-1 of every chunk this one's scores depend on
                    # was queued as filler in earlier positions; drain
                    # any remainder of this chunk's own Q/K granules.
                    drain_through((tcx, 'qk'))
                    p1u = []
                    for nxt in P1_AT_POS[pos]:
                        p1u.extend(make_phase1_units(nxt))
                    p4u = finish_pending()  # [norm, m0..m7]
                    # interleave: the norm/out-proj units wait on a
                    # scalar->PE->DVE chain; dependency-free phase-1
                    # granules between them keep the PE FIFO head from
                    # blocking at chunk boundaries (HAM re-throttles).
                    merged = list(p1u[0:2])
                    i1 = 2
                    for u4 in p4u:
                        merged.append(u4)
                        merged.extend(p1u[i1:i1 + 2])
                        i1 += 2
                    merged.extend(p1u[i1:])
                    filler.extend(merged)
                    # emit through norm before the k-loop so its Z-row
                    # copy lands on the ScalarE queue ahead of this
                    # chunk's first exp.
                    pop_fillers(3 if p1u else 1)

                # both heads' AV accumulators in one 2-bank tile:
                # head h at columns [h*TC, (h+1)*TC); Z row at partition 64
                otp = pspool.tile([65, 2 * TC], FP, tag="av", bufs=1,
                                  name=f"otp{tcx}")
                pts = {}

                def emit_av(j, b=b, n=n, otp=otp, pts=pts):
                    kgj = b * 16 + j
                    pt, w = pts[j]
                    for h in range(2):
                        nc.tensor.matmul(
                            otp[:, h * TC + TC - w:h * TC + TC],
                            vaug[:, kgj * 130 + h * 65:
                                 kgj * 130 + h * 65 + 65],
                            pt[:, h * w:(h + 1) * w],
                            start=(j == 0), stop=(j == n - 1),
                            skip_group_check=True,
                        )

                for kt in range(n):
                    kg = b * 16 + kt
                    # diagonal k-tile v only attends to q >= v*128: shrink
                    # the free width of scores/exp/AV accordingly
                    v = kt - (n - 4)
                    w = TC - 128 * v if v > 0 else TC
                    s = pspool.tile([128, 2 * TC], FP, tag="s", bufs=2,
                                    name=f"s{tcx}_{kt}")
                    for h in range(2):
                        # head h stays at fixed offset h*TC: a matmul
                        # destination must not cross a PSUM bank boundary
                        nc.tensor.matmul(
                            s[:, h * TC:h * TC + w],
                            KT[h * 64:(h + 1) * 64, kg * 128:(kg + 1) * 128],
                            QT[h * 64:(h + 1) * 64, t0 + TC - w:t0 + TC],
                            start=True, stop=True,
                        )
                    # V granules of this chunk must land before the
                    # diagonal AV matmuls need vaug
                    if kt >= max(n - 6, 0):
                        drain_through((tcx, 'v'))
                    # adaptive filler drain across remaining k-slots
                    slots = n - kt
                    pop_fillers(-(-len(filler) // slots) if filler else 0)
                    pt = sbpool.tile([128, 2 * TC], BF, tag="pt", bufs=8,
                                     name=f"pt{tcx}_{kt}")
                    nc.scalar.activation(
                        pt[:, 0:2 * w].rearrange("p (h q) -> p h q", h=2),
                        s[:].rearrange("p (h q) -> p h q", h=2)[:, :, 0:w],
                        ACT.Exp, scale=0.125)
                    if kt >= n - 4:
                        # multiplicative causal mask on the first 128 q
                        # columns of each head's block (the in-tile triangle)
                        nc.vector.tensor_mul(
                            pt[:, 0:2 * w].rearrange(
                                "p (h q) -> p h q", h=2)[:, :, 0:128],
                            pt[:, 0:2 * w].rearrange(
                                "p (h q) -> p h q", h=2)[:, :, 0:128],
                            masks_sb[:].rearrange(
                                "p (h q) -> p h q", h=2),
                        )
                    pts[kt] = (pt, w)
                    if kt >= AV_DELAY:
                        emit_av(kt - AV_DELAY)
                pending = (tcx, b, n, otp, pts, emit_av)

            pop_fillers(len(filler))
            for _, u in finish_pending():
                u()
    nc.compile()
    return nc


def make_in_maps(x, w_qkv, b_qkv, w_out):
    x = np.ascontiguousarray(np.asarray(x, np.float32).reshape(BT, C))
    xT = np.ascontiguousarray(x.T).astype(ml_dtypes.bfloat16)
    w_qkv = np.asarray(w_qkv, np.float32)
    b_qkv = np.asarray(b_qkv, np.float32)
    w_out = np.asarray(w_out, np.float32)

    # In-tile causal triangle, duplicated per head: masks[p, h*128 + q] =
    # 1 if p <= q else 0 (multiplies the exp output: exact causal zeroing)
    kk = np.arange(128)[:, None, None]
    qq = np.arange(128)[None, None, :]
    mask = np.where(kk <= qq, 1.0, 0.0) * np.ones((1, 2, 1))
    mask = np.ascontiguousarray(
        mask.reshape(128, 256)).astype(ml_dtypes.bfloat16)

    # wc[p, k*384 + g*128 + j] = w_qkv[k*128 + p, g*1024 + c0 + j]
    w4 = w_qkv.reshape(8, 128, 3, 1024)

    in_maps = []
    for c in range(NCORES):
        c0 = c * 128
        wcs = np.ascontiguousarray(
            w4[:, :, :, c0:c0 + 128].transpose(1, 0, 2, 3).reshape(128, 3072)
        ).astype(ml_dtypes.bfloat16)
        bq = np.ascontiguousarray(
            b_qkv.reshape(3, 1024)[:, c0:c0 + 128].T)
        bv = np.ascontiguousarray(
            b_qkv[2 * 1024 + c0:2 * 1024 + c0 + 128].reshape(1, 128)
        ).astype(ml_dtypes.bfloat16)
        in_maps.append({
            "xt": xT,
            "wc": wcs,
            "wout": np.ascontiguousarray(
                w_out[c0:c0 + 128, :]).astype(ml_dtypes.bfloat16),
            "bqkv": bq,
            "bvrow": bv,
            "masks": mask,
            "onesr": np.ones((1, 64), np.float32),
            "onesb": np.ones((1, 128), ml_dtypes.bfloat16),
        })
    return in_maps


_NC_CACHE = None


def kernel(x, w_qkv, b_qkv, w_out, b_out):
    global _NC_CACHE, LAST_RESULTS
    if _NC_CACHE is None:
        _NC_CACHE = build_nc()
    nc = _NC_CACHE

    in_maps = make_in_maps(x, w_qkv, b_qkv, w_out)

    res = run_bass_kernel_spmd(
        nc, in_maps, list(range(NCORES)),
        trace=bool(os.environ.get("BASS_TRACE")),
    )
    LAST_RESULTS = res

    acc = np.zeros((C, BT), np.float64)
    for out_map in res.results:
        # yt[i*128 + p, m*512 + t] -> y_partial[m*128 + p, i*512 + t]
        yp = out_map["yt"].astype(np.float32)
        yp = yp.reshape(8, 128, 8, 512).transpose(2, 1, 0, 3).reshape(C, BT)
        acc += yp
    y = acc.T.astype(np.float32) + np.asarray(b_out, np.float32)[None, :]
    return y.reshape(B, T, C)


# revision 26
# speedup vs baseline: 1.0089x; 1.0089x over previous
"""Multi-head causal attention (B=2, T=2048, C=1024, H=16) on 8 Trainium2
NeuronCores, tensor-parallel over heads (2 heads per core).

v2: trace-driven rework of the v1 software-pipelined kernel.

  - V is produced token-major directly in phase-1 (lhsT = x token tiles,
    rhs = w_v columns): the 32 PE transposes, their identity loads and
    the vt/tp DVE staging copies are gone, along with the DVE-queue
    head-of-line tangle they caused at chunk boundaries (exp -> mask ->
    vt-copy -> transpose -> AV cycles cost ~4.6us stalls + HAM cold).
  - both heads' AV accumulators live in one [65, 2*TC] PSUM tile
    (adjacent banks): one DVE copy grabs both Z rows, reciprocal runs
    once on [1, 2*TC], and the PE bch matmuls broadcast the reciprocal
    (the per-head [64, TC] reciprocals are gone).
  - ScalarE does nothing but exp (it was 57% busy with copies): all
    PSUM evacuations (phase-1 bias adds, phase-4 out-proj copies, norm)
    run on DVE; V bias is folded into phase-1 as a rank-1 ones matmul.
  - chunk order ends with a 4-ktile chunk (b1 qc=0) so the big second-
    to-last chunk's out-projection overlaps that chunk's k-loop instead
    of draining after everything else; output DMA is per-m (128 KB).

Host-side layouts are pre-tiled for contiguous DMA rows; the 8 partial
[C, BT] outputs are summed (and bias added) on the host.
"""

import os
import sys
from collections import deque

for _p in ("/opt/trn_rl_repo", "/root/.axon_site/_ro/trn_rl_repo"):
    if os.path.isdir(_p) and _p not in sys.path:
        sys.path.insert(0, _p)

import ml_dtypes
import numpy as np

import concourse.bacc as bacc
import concourse.bass as bass
import concourse.mybir as mybir
import concourse.tile as tile
from concourse.bass_utils import run_bass_kernel_spmd

B, T, C, H, D = 2, 2048, 1024, 16, 64
NCORES = 8
BT = B * T                      # 4096 flattened tokens
TC = 512                        # token chunk (matmul free dim)
NTC = BT // TC                  # 8 token chunks
FP = mybir.dt.float32
FPR = mybir.dt.float32r
BF = mybir.dt.bfloat16
ACT = mybir.ActivationFunctionType
AV_DELAY = 3                    # k-tiles the AV matmul trails the scores

# chunk processing order: logical chunk ids (b*4 + qc).
CHUNK_ORDER = [0, 1, 2, 3, 4, 5, 6, 7]
# phase-1 filler emitted during each position (chunk ids); chunk 0's
# phase-1 runs inline before the loop.
P1_AT_POS = {0: [1], 1: [2], 2: [3], 3: [4], 4: [5], 5: [6], 6: [7], 7: []}

LAST_RESULTS = None             # stashed BassKernelResults for test harness


def build_nc():
    nc = bacc.Bacc(None, target_bir_lowering=False, debug=False)

    xt = nc.declare_dram_parameter("xt", [C, BT], BF, isOutput=False)
    wc = nc.declare_dram_parameter("wc", [128, 3072], BF, isOutput=False)
    wout = nc.declare_dram_parameter("wout", [128, C], BF, isOutput=False)
    bqkv = nc.declare_dram_parameter("bqkv", [128, 3], FP, isOutput=False)
    bvrow = nc.declare_dram_parameter("bvrow", [1, 128], BF, isOutput=False)
    masks = nc.declare_dram_parameter("masks", [128, 256], BF, isOutput=False)
    onesr = nc.declare_dram_parameter("onesr", [1, 64], FP, isOutput=False)
    onesb = nc.declare_dram_parameter("onesb", [1, 128], BF, isOutput=False)
    # yt[(m*8 + i)*128 + p, t] = y_partial[m*128 + p, i*512 + t]:
    # m-outer layout makes each per-m store a contiguous 128 KB block
    # (8 KB bursts) instead of 1 KB strided row pieces
    yt = nc.declare_dram_parameter("yt", [64 * 128, TC], BF, isOutput=True)

    with tile.TileContext(nc) as tc:
        with (
            tc.tile_pool(name="const", bufs=1) as cpool,
            tc.tile_pool(name="big", bufs=1) as bigpool,
            tc.tile_pool(name="sb", bufs=2) as sbpool,
            tc.tile_pool(name="ps", bufs=2, space="PSUM") as pspool,
        ):
            # ---- constants; DMA order = startup order ----
            # wc split per k-slice so the first phase-1 matmul only waits
            # on a 96 KB transfer; chunk-0 x columns land first. DMAs are
            # spread over sync+gpsimd dynamic queues ONLY: a trigger on
            # the ScalarE queue blocks exp behind DMA-completion waits
            # (measured 5 us PE stalls in the first third of the kernel).
            dma_engs = (nc.sync, nc.gpsimd)
            ndma = 0

            def dma(out, in_):
                nonlocal ndma
                dma_engs[ndma % len(dma_engs)].dma_start(out=out, in_=in_)
                ndma += 1

            wc_sb = cpool.tile([128, 3072], BF)
            xks = []
            for k in range(8):
                xk = cpool.tile([128, BT], BF, name=f"xk{k}")
                xks.append(xk)
            for k in range(8):
                dma(wc_sb[:, k * 384:(k + 1) * 384],
                    wc[:, k * 384:(k + 1) * 384])
                dma(xks[k][:, 0:TC], xt[k * 128:(k + 1) * 128, 0:TC])
            # small constants BEFORE the bulk x stream: chunk 0's mask
            # multiply head-blocked the DVE FIFO ~7 us waiting for a
            # masks tile queued behind 7 MB of x columns.
            bq_sb = cpool.tile([128, 3], FP)
            dma(bq_sb[:], bqkv[:, :])
            bv_sb = cpool.tile([1, 128], BF)
            dma(bv_sb[:], bvrow[:, :])
            onesb_sb = cpool.tile([1, 128], BF)
            dma(onesb_sb[:], onesb[:, :])
            masks_sb = cpool.tile([128, 256], BF)
            dma(masks_sb[:], masks[:, :])
            onesr_sb = cpool.tile([1, 64], FPR)
            dma(onesr_sb[:], onesr.bitcast(FPR)[:, :])
            wout_sb = cpool.tile([128, C], BF)
            dma(wout_sb[:], wout[:, :])
            # remaining x columns, chunk-pair major so chunk c's slice
            # lands before phase-1(c) needs it
            for c0 in range(TC, BT, 2 * TC):
                c1 = min(c0 + 2 * TC, BT)
                for k in range(8):
                    dma(xks[k][:, c0:c1], xt[k * 128:(k + 1) * 128, c0:c1])

            # ---- HAM warm-up: keep the PE busy on scratch data while the
            # initial DMAs land, so real work starts at 2.4 GHz ----
            dummy = cpool.tile([128, 128], BF, name="dummy")
            nc.vector.memset(dummy[:], 0.0)
            for _ in range(56):
                dps = pspool.tile([128, 128], FP, tag="m", bufs=2,
                                  name="dps")
                nc.tensor.matmul(dps[:], dummy[:], dummy[:],
                                 start=True, stop=True)

            # ---- persistent intermediates ----
            QT = bigpool.tile([128, BT], BF)
            KT = bigpool.tile([128, BT], BF)
            # V in [token, dim] layout, 130 cols per 128-token block:
            # [V_h0 (64) | ones | V_h1 (64) | ones]; memset once puts the
            # ones columns in place, the phase-1 V copy overwrites the rest.
            vaug = bigpool.tile([128, 32 * 130], BF)
            nc.vector.memset(vaug[:], 1.0)

            p1state = {}

            def make_phase1_units(i):
                """QKV projection for chunk i in ~2-matmul granules.
                Q/K land dim-major in QT/KT; V lands token-major in a
                PSUM tile (lhsT = x token tiles) and is copied into vaug
                with one DVE op — no PE transposes."""
                t0 = i * TC

                def qk_granule(g, k0):
                    def emit():
                        if k0 == 0:
                            p1state[(i, g)] = pspool.tile(
                                [128, TC], FP, tag="m", bufs=2,
                                name=f"p1q{i}g{g}")
                        ps = p1state[(i, g)]
                        for k in (k0, k0 + 1):
                            nc.tensor.matmul(
                                ps[:],
                                wc_sb[:, k * 384 + g * 128:
                                      k * 384 + (g + 1) * 128],
                                xks[k][:, t0:t0 + TC],
                                start=(k == 0), stop=(k == 7),
                            )
                        if k0 == 6:
                            del p1state[(i, g)]
                            nc.vector.tensor_scalar_add(
                                out=(QT, KT)[g][:, t0:t0 + TC], in0=ps[:],
                                scalar1=bq_sb[:, g:g + 1])
                    return emit

                def v_granule(j):
                    # one token-tile per unit (8 matmuls + bias): long
                    # back-to-back PE runs keep the LDW pipeline primed
                    def emit():
                        if j == 0:
                            p1state[(i, 'v')] = pspool.tile(
                                [128, TC], FP, tag="m", bufs=2,
                                name=f"p1v{i}")
                        ps = p1state[(i, 'v')]
                        for k in range(8):
                            nc.tensor.matmul(
                                ps[:, j * 128:(j + 1) * 128],
                                xks[k][:, t0 + j * 128:t0 + (j + 1) * 128],
                                wc_sb[:, k * 384 + 256:k * 384 + 384],
                                start=(k == 0), stop=False,
                            )
                        # fold the V bias in as a rank-1 ones matmul
                        nc.tensor.matmul(
                            ps[:, j * 128:(j + 1) * 128],
                            onesb_sb[0:1, :], bv_sb[0:1, :],
                            start=False, stop=True,
                        )
                    return emit

                def v_copy():
                    ps = p1state.pop((i, 'v'))
                    nc.vector.tensor_copy(
                        vaug[:].rearrange(
                            "p (j a c) -> p j a c", a=2, c=65)[
                            :, i * 4:(i + 1) * 4, :, 0:64],
                        ps[:].rearrange("p (j a c) -> p j a c", j=4, c=64),
                    )

                qk = [((i, 'qk'), qk_granule(g, k0)) for g in range(2)
                      for k0 in range(0, 8, 2)]
                v = [((i, 'v'), v_granule(j)) for j in range(4)]
                v.append(((i, 'v'), v_copy))
                return qk + v

            filler = deque()

            def pop_fillers(k):
                for _ in range(k):
                    if filler:
                        filler.popleft()[1]()

            def drain_through(key):
                """Pop until no unit with the given key remains (they form a
                contiguous run in FIFO order)."""
                while any(u[0] == key for u in filler):
                    filler.popleft()[1]()

            # state deferred from the previous chunk
            pending = None  # (i, b, n, otp, pts, emit_av)

            def finish_pending():
                """Tail AVs for the previous chunk (emitted directly);
                normalization + phase-4 are returned as filler units."""
                nonlocal pending
                if pending is None:
                    return []
                pi, pb, pn, potp, ppts, pemit_av = pending
                pending = None
                for j in range(max(pn - AV_DELAY, 0), pn):
                    pemit_av(j)
                state = {}

                def norm_unit():
                    # both heads' Z rows sit at partition 64 of the two
                    # adjacent banks of potp: one f32r copy, PE broadcast
                    # (bch), then reciprocal + multiply per head on DVE.
                    rz = sbpool.tile([1, 2 * TC], FPR, tag="rz", bufs=2,
                                     name=f"rz{pi}")
                    with nc.allow_low_precision(reason="sums f32r"):
                        nc.scalar.copy(rz[:], potp[64:65, :])
                    ot = sbpool.tile([128, TC], BF, tag="ot", bufs=2,
                                     name=f"ot{pi}")
                    for h in range(2):
                        bchh = pspool.tile([64, TC], FP, tag="m", bufs=2,
                                           name=f"bch{h}_{pi}")
                        nc.tensor.matmul(bchh[:], onesr_sb[0:1, :],
                                         rz[0:1, h * TC:(h + 1) * TC],
                                         start=True, stop=True)
                        bcsh = sbpool.tile([64, TC], FP, tag=f"bcs{h}",
                                           bufs=2, name=f"bcs{h}_{pi}")
                        nc.vector.reciprocal_approx_fast(out=bcsh[:],
                                                         in_=bchh[:])
                        nc.vector.tensor_mul(
                            ot[h * 64:(h + 1) * 64, :],
                            potp[0:64, h * TC:(h + 1) * TC], bcsh[:])
                    state['ot'] = ot
                    state['ys'] = sbpool.tile([128, BT], BF, tag="ys",
                                              bufs=2, name=f"ys{pi}")

                def m_unit(m, last):
                    # PSUM evacuation engine: mostly DVE; ScalarE takes a
                    # couple per chunk (it has slack between exps), and on
                    # the final chunk (no exp left) strict alternation
                    # halves the copy-bound tail chain.
                    if last:
                        ceng = (nc.vector.tensor_copy,
                                nc.scalar.copy)[m % 2]
                    else:
                        ceng = (nc.vector.tensor_copy if m % 4 != 3
                                else nc.scalar.copy)

                    def emit():
                        yp = pspool.tile([128, TC], FP, tag="m", bufs=2,
                                         name=f"yp{pi}_{m}")
                        nc.tensor.matmul(
                            yp[:], wout_sb[:, m * 128:(m + 1) * 128],
                            state['ot'][:], start=True, stop=True)
                        ys = state['ys']
                        ceng(ys[:, m * TC:(m + 1) * TC], yp[:])
                        r0 = (m * 8 + pi) * 128
                        (nc.sync, nc.gpsimd)[m % 2].dma_start(
                            out=yt[r0:r0 + 128, :],
                            in_=ys[:, m * TC:(m + 1) * TC])
                    return emit

                last = pi == CHUNK_ORDER[-1]
                return [((pi, 'p4'), norm_unit)] + \
                    [((pi, 'p4'), m_unit(m, last)) for m in range(8)]

            for pos, tcx in enumerate(CHUNK_ORDER):
                b, qc = divmod(tcx, 4)
                t0 = tcx * TC
                n = 4 * (qc + 1)

                if pos == 0:
                    for _, u in make_phase1_units(tcx):
                        u()
                    for nxt in P1_AT_POS[pos]:
                        filler.extend(make_phase1_units(nxt))
                else:
                    # phase-1 of every chunk this one's scores depend on
                    # was queued as filler in earlier positions; drain
                    # any remainder of this chunk's own Q/K granules.
                    drain_through((tcx, 'qk'))
                    p1u = []
                    for nxt in P1_AT_POS[pos]:
                        p1u.extend(make_phase1_units(nxt))
                    p4u = finish_pending()  # [norm, m0..m7]
                    # interleave: the norm/out-proj units wait on a
                    # scalar->PE->DVE chain; dependency-free phase-1
                    # granules between them keep the PE FIFO head from
                    # blocking at chunk boundaries (HAM re-throttles).
                    merged = list(p1u[0:2])
                    i1 = 2
                    for u4 in p4u:
                        merged.append(u4)
                        merged.extend(p1u[i1:i1 + 2])
                        i1 += 2
                    merged.extend(p1u[i1:])
                    filler.extend(merged)
                    # emit through norm before the k-loop so its Z-row
                    # copy lands on the ScalarE queue ahead of this
                    # chunk's first exp.
                    pop_fillers(3 if p1u else 1)

                # both heads' AV accumulators in one 2-bank tile:
                # head h at columns [h*TC, (h+1)*TC); Z row at partition 64
                otp = pspool.tile([65, 2 * TC], FP, tag="av", bufs=1,
                                  name=f"otp{tcx}")
                pts = {}

                def emit_av(j, b=b, n=n, otp=otp, pts=pts):
                    kgj = b * 16 + j
                    pt, w = pts[j]
                    for h in range(2):
                        nc.tensor.matmul(
                            otp[:, h * TC + TC - w:h * TC + TC],
                            vaug[:, kgj * 130 + h * 65:
                                 kgj * 130 + h * 65 + 65],
                            pt[:, h * w:(h + 1) * w],
                            start=(j == 0), stop=(j == n - 1),
                            skip_group_check=True,
                        )

                for kt in range(n):
                    kg = b * 16 + kt
                    # diagonal k-tile v only attends to q >= v*128: shrink
                    # the free width of scores/exp/AV accordingly
                    v = kt - (n - 4)
                    w = TC - 128 * v if v > 0 else TC
                    s = pspool.tile([128, 2 * TC], FP, tag="s", bufs=2,
                                    name=f"s{tcx}_{kt}")
                    for h in range(2):
                        # head h stays at fixed offset h*TC: a matmul
                        # destination must not cross a PSUM bank boundary
                        nc.tensor.matmul(
                            s[:, h * TC:h * TC + w],
                            KT[h * 64:(h + 1) * 64, kg * 128:(kg + 1) * 128],
                            QT[h * 64:(h + 1) * 64, t0 + TC - w:t0 + TC],
                            start=True, stop=True,
                        )
                    # V granules of this chunk must land before the
                    # diagonal AV matmuls need vaug
                    if kt >= max(n - 6, 0):
                        drain_through((tcx, 'v'))
                    # adaptive filler drain across remaining k-slots
                    slots = n - kt
                    pop_fillers(-(-len(filler) // slots) if filler else 0)
                    pt = sbpool.tile([128, 2 * TC], BF, tag="pt", bufs=8,
                                     name=f"pt{tcx}_{kt}")
                    nc.scalar.activation(
                        pt[:, 0:2 * w].rearrange("p (h q) -> p h q", h=2),
                        s[:].rearrange("p (h q) -> p h q", h=2)[:, :, 0:w],
                        ACT.Exp, scale=0.125)
                    if kt >= n - 4:
                        # multiplicative causal mask on the first 128 q
                        # columns of each head's block (the in-tile triangle)
                        nc.vector.tensor_mul(
                            pt[:, 0:2 * w].rearrange(
                                "p (h q) -> p h q", h=2)[:, :, 0:128],
                            pt[:, 0:2 * w].rearrange(
                                "p (h q) -> p h q", h=2)[:, :, 0:128],
                            masks_sb[:].rearrange(
                                "p (h q) -> p h q", h=2),
                        )
                    pts[kt] = (pt, w)
                    if kt >= AV_DELAY:
                        emit_av(kt - AV_DELAY)
                pending = (tcx, b, n, otp, pts, emit_av)

            pop_fillers(len(filler))
            for _, u in finish_pending():
                u()
    nc.compile()
    return nc


def make_in_maps(x, w_qkv, b_qkv, w_out):
    x = np.ascontiguousarray(np.asarray(x, np.float32).reshape(BT, C))
    xT = np.ascontiguousarray(x.T).astype(ml_dtypes.bfloat16)
    w_qkv = np.asarray(w_qkv, np.float32)
    b_qkv = np.asarray(b_qkv, np.float32)
    w_out = np.asarray(w_out, np.float32)

    # In-tile causal triangle, duplicated per head: masks[p, h*128 + q] =
    # 1 if p <= q else 0 (multiplies the exp output: exact causal zeroing)
    kk = np.arange(128)[:, None, None]
    qq = np.arange(128)[None, None, :]
    mask = np.where(kk <= qq, 1.0, 0.0) * np.ones((1, 2, 1))
    mask = np.ascontiguousarray(
        mask.reshape(128, 256)).astype(ml_dtypes.bfloat16)

    # wc[p, k*384 + g*128 + j] = w_qkv[k*128 + p, g*1024 + c0 + j]
    w4 = w_qkv.reshape(8, 128, 3, 1024)

    in_maps = []
    for c in range(NCORES):
        c0 = c * 128
        wcs = np.ascontiguousarray(
            w4[:, :, :, c0:c0 + 128].transpose(1, 0, 2, 3).reshape(128, 3072)
        ).astype(ml_dtypes.bfloat16)
        bq = np.ascontiguousarray(
            b_qkv.reshape(3, 1024)[:, c0:c0 + 128].T)
        bv = np.ascontiguousarray(
            b_qkv[2 * 1024 + c0:2 * 1024 + c0 + 128].reshape(1, 128)
        ).astype(ml_dtypes.bfloat16)
        in_maps.append({
            "xt": xT,
            "wc": wcs,
            "wout": np.ascontiguousarray(
                w_out[c0:c0 + 128, :]).astype(ml_dtypes.bfloat16),
            "bqkv": bq,
            "bvrow": bv,
            "masks": mask,
            "onesr": np.ones((1, 64), np.float32),
            "onesb": np.ones((1, 128), ml_dtypes.bfloat16),
        })
    return in_maps


_NC_CACHE = None


def kernel(x, w_qkv, b_qkv, w_out, b_out):
    global _NC_CACHE, LAST_RESULTS
    if _NC_CACHE is None:
        _NC_CACHE = build_nc()
    nc = _NC_CACHE

    in_maps = make_in_maps(x, w_qkv, b_qkv, w_out)

    res = run_bass_kernel_spmd(
        nc, in_maps, list(range(NCORES)),
        trace=bool(os.environ.get("BASS_TRACE")),
    )
    LAST_RESULTS = res

    acc = np.zeros((C, BT), np.float64)
    for out_map in res.results:
        # yt[(m*8 + i)*128 + p, t] -> y_partial[m*128 + p, i*512 + t]
        yp = out_map["yt"].astype(np.float32)
        yp = yp.reshape(8, 8, 128, 512).transpose(0, 2, 1, 3).reshape(C, BT)
        acc += yp
    y = acc.T.astype(np.float32) + np.asarray(b_out, np.float32)[None, :]
    return y.reshape(B, T, C)


# revision 27
# speedup vs baseline: 1.0128x; 1.0039x over previous
"""Multi-head causal attention (B=2, T=2048, C=1024, H=16) on 8 Trainium2
NeuronCores, tensor-parallel over heads (2 heads per core).

v10: trace-driven rework of the v1 software-pipelined kernel
(215 us traced -> 189 us traced; ~8% HW-exec win, throttle time
60 us -> 35 us, ScalarE busy 126 us -> 98 us).

  - V is produced token-major directly in phase-1 (lhsT = x token
    tiles, rhs = w_v columns, 8 matmuls + rank-1 bias per token tile):
    the 32 PE transposes, their identity loads and the vt/tp DVE
    staging copies are gone, along with the DVE-queue head-of-line
    tangle they caused at chunk boundaries.
  - both heads' AV accumulators live in one [65, 2*TC] PSUM tile
    (adjacent banks): one ScalarE f32r copy grabs both Z rows, then
    PE bch broadcast + DVE reciprocal/multiply per head.
  - ScalarE is essentially exp-only: PSUM evacuations run on DVE
    except 2 of 8 out-proj copies per chunk (and alternating on the
    final chunk, where exp is done).  DMA triggers NEVER go on the
    ScalarE queue: a trigger there blocks exp behind DMA-completion
    waits (measured 5-7 us PE stalls).
  - small constants (masks/wout/onesr) are DMAed before the bulk x
    stream: chunk 0's mask multiply head-blocked the DVE FIFO ~7 us
    waiting for a masks tile queued behind 7 MB of x columns.
  - at chunk boundaries the previous chunk's norm/out-proj units are
    interleaved between dependency-free phase-1 granules (they wait on
    a scalar->PE->DVE chain and would head-block the PE FIFO; HAM
    re-throttles after ~600 ns gap clusters), and norm is emitted
    before the k-loop so its Z-row copy precedes this chunk's first
    exp in the ScalarE FIFO.
  - output layout yt[(m*8+i)*128+p, t]: each out-proj m-block is a
    contiguous 128 KB DRAM store (8 KB bursts) issued right after its
    evacuation copy, spread over the sync/gpsimd queues.

Host-side layouts are pre-tiled for contiguous DMA rows; the 8 partial
[C, BT] outputs are summed (and bias added) on the host.
"""

import os
import sys
from collections import deque

for _p in ("/opt/trn_rl_repo", "/root/.axon_site/_ro/trn_rl_repo"):
    if os.path.isdir(_p) and _p not in sys.path:
        sys.path.insert(0, _p)

import ml_dtypes
import numpy as np

import concourse.bacc as bacc
import concourse.bass as bass
import concourse.mybir as mybir
import concourse.tile as tile
from concourse.bass_utils import run_bass_kernel_spmd

B, T, C, H, D = 2, 2048, 1024, 16, 64
NCORES = 8
BT = B * T                      # 4096 flattened tokens
TC = 512                        # token chunk (matmul free dim)
NTC = BT // TC                  # 8 token chunks
FP = mybir.dt.float32
FPR = mybir.dt.float32r
BF = mybir.dt.bfloat16
ACT = mybir.ActivationFunctionType
AV_DELAY = 3                    # k-tiles the AV matmul trails the scores

# chunk processing order: logical chunk ids (b*4 + qc).
CHUNK_ORDER = [0, 1, 2, 3, 4, 5, 6, 7]
# phase-1 filler emitted during each position (chunk ids); chunk 0's
# phase-1 runs inline before the loop.
P1_AT_POS = {0: [1], 1: [2], 2: [3], 3: [4], 4: [5], 5: [6], 6: [7], 7: []}

LAST_RESULTS = None             # stashed BassKernelResults for test harness


def build_nc():
    nc = bacc.Bacc(None, target_bir_lowering=False, debug=False)

    xt = nc.declare_dram_parameter("xt", [C, BT], BF, isOutput=False)
    wc = nc.declare_dram_parameter("wc", [128, 3072], BF, isOutput=False)
    wout = nc.declare_dram_parameter("wout", [128, C], BF, isOutput=False)
    bqkv = nc.declare_dram_parameter("bqkv", [128, 3], FP, isOutput=False)
    bvrow = nc.declare_dram_parameter("bvrow", [1, 128], BF, isOutput=False)
    masks = nc.declare_dram_parameter("masks", [128, 256], BF, isOutput=False)
    onesr = nc.declare_dram_parameter("onesr", [1, 64], FP, isOutput=False)
    onesb = nc.declare_dram_parameter("onesb", [1, 128], BF, isOutput=False)
    # yt[(m*8 + i)*128 + p, t] = y_partial[m*128 + p, i*512 + t]:
    # m-outer layout makes each per-m store a contiguous 128 KB block
    # (8 KB bursts) instead of 1 KB strided row pieces
    yt = nc.declare_dram_parameter("yt", [64 * 128, TC], BF, isOutput=True)

    with tile.TileContext(nc) as tc:
        with (
            tc.tile_pool(name="const", bufs=1) as cpool,
            tc.tile_pool(name="big", bufs=1) as bigpool,
            tc.tile_pool(name="sb", bufs=2) as sbpool,
            tc.tile_pool(name="ps", bufs=2, space="PSUM") as pspool,
        ):
            # ---- constants; DMA order = startup order ----
            # wc split per k-slice so the first phase-1 matmul only waits
            # on a 96 KB transfer; chunk-0 x columns land first. DMAs are
            # spread over sync+gpsimd dynamic queues ONLY: a trigger on
            # the ScalarE queue blocks exp behind DMA-completion waits
            # (measured 5 us PE stalls in the first third of the kernel).
            dma_engs = (nc.sync, nc.gpsimd)
            ndma = 0

            def dma(out, in_):
                nonlocal ndma
                dma_engs[ndma % len(dma_engs)].dma_start(out=out, in_=in_)
                ndma += 1

            wc_sb = cpool.tile([128, 3072], BF)
            xks = []
            for k in range(8):
                xk = cpool.tile([128, BT], BF, name=f"xk{k}")
                xks.append(xk)
            for k in range(8):
                dma(wc_sb[:, k * 384:(k + 1) * 384],
                    wc[:, k * 384:(k + 1) * 384])
                dma(xks[k][:, 0:TC], xt[k * 128:(k + 1) * 128, 0:TC])
            # small constants BEFORE the bulk x stream: chunk 0's mask
            # multiply head-blocked the DVE FIFO ~7 us waiting for a
            # masks tile queued behind 7 MB of x columns.
            bq_sb = cpool.tile([128, 3], FP)
            dma(bq_sb[:], bqkv[:, :])
            bv_sb = cpool.tile([1, 128], BF)
            dma(bv_sb[:], bvrow[:, :])
            onesb_sb = cpool.tile([1, 128], BF)
            dma(onesb_sb[:], onesb[:, :])
            masks_sb = cpool.tile([128, 256], BF)
            dma(masks_sb[:], masks[:, :])
            onesr_sb = cpool.tile([1, 64], FPR)
            dma(onesr_sb[:], onesr.bitcast(FPR)[:, :])
            wout_sb = cpool.tile([128, C], BF)
            dma(wout_sb[:], wout[:, :])
            # remaining x columns, chunk-pair major so chunk c's slice
            # lands before phase-1(c) needs it
            for c0 in range(TC, BT, 2 * TC):
                c1 = min(c0 + 2 * TC, BT)
                for k in range(8):
                    dma(xks[k][:, c0:c1], xt[k * 128:(k + 1) * 128, c0:c1])

            # ---- HAM warm-up: keep the PE busy on scratch data while the
            # initial DMAs land, so real work starts at 2.4 GHz ----
            dummy = cpool.tile([128, 128], BF, name="dummy")
            nc.vector.memset(dummy[:], 0.0)
            for _ in range(56):
                dps = pspool.tile([128, 128], FP, tag="m", bufs=2,
                                  name="dps")
                nc.tensor.matmul(dps[:], dummy[:], dummy[:],
                                 start=True, stop=True)

            # ---- persistent intermediates ----
            QT = bigpool.tile([128, BT], BF)
            KT = bigpool.tile([128, BT], BF)
            # V in [token, dim] layout, 130 cols per 128-token block:
            # [V_h0 (64) | ones | V_h1 (64) | ones]; memset once puts the
            # ones columns in place, the phase-1 V copy overwrites the rest.
            vaug = bigpool.tile([128, 32 * 130], BF)
            nc.vector.memset(vaug[:], 1.0)

            p1state = {}

            def make_phase1_units(i):
                """QKV projection for chunk i in ~2-matmul granules.
                Q/K land dim-major in QT/KT; V lands token-major in a
                PSUM tile (lhsT = x token tiles) and is copied into vaug
                with one DVE op — no PE transposes."""
                t0 = i * TC

                def qk_granule(g, k0):
                    def emit():
                        if k0 == 0:
                            p1state[(i, g)] = pspool.tile(
                                [128, TC], FP, tag="m", bufs=2,
                                name=f"p1q{i}g{g}")
                        ps = p1state[(i, g)]
                        for k in (k0, k0 + 1):
                            nc.tensor.matmul(
                                ps[:],
                                wc_sb[:, k * 384 + g * 128:
                                      k * 384 + (g + 1) * 128],
                                xks[k][:, t0:t0 + TC],
                                start=(k == 0), stop=(k == 7),
                            )
                        if k0 == 6:
                            del p1state[(i, g)]
                            nc.vector.tensor_scalar_add(
                                out=(QT, KT)[g][:, t0:t0 + TC], in0=ps[:],
                                scalar1=bq_sb[:, g:g + 1])
                    return emit

                def v_granule(j):
                    # one token-tile per unit (8 matmuls + bias): long
                    # back-to-back PE runs keep the LDW pipeline primed
                    def emit():
                        if j == 0:
                            p1state[(i, 'v')] = pspool.tile(
                                [128, TC], FP, tag="m", bufs=2,
                                name=f"p1v{i}")
                        ps = p1state[(i, 'v')]
                        for k in range(8):
                            nc.tensor.matmul(
                                ps[:, j * 128:(j + 1) * 128],
                                xks[k][:, t0 + j * 128:t0 + (j + 1) * 128],
                                wc_sb[:, k * 384 + 256:k * 384 + 384],
                                start=(k == 0), stop=False,
                            )
                        # fold the V bias in as a rank-1 ones matmul
                        nc.tensor.matmul(
                            ps[:, j * 128:(j + 1) * 128],
                            onesb_sb[0:1, :], bv_sb[0:1, :],
                            start=False, stop=True,
                        )
                    return emit

                def v_copy():
                    ps = p1state.pop((i, 'v'))
                    nc.vector.tensor_copy(
                        vaug[:].rearrange(
                            "p (j a c) -> p j a c", a=2, c=65)[
                            :, i * 4:(i + 1) * 4, :, 0:64],
                        ps[:].rearrange("p (j a c) -> p j a c", j=4, c=64),
                    )

                qk = [((i, 'qk'), qk_granule(g, k0)) for g in range(2)
                      for k0 in range(0, 8, 2)]
                v = [((i, 'v'), v_granule(j)) for j in range(4)]
                v.append(((i, 'v'), v_copy))
                return qk + v

            filler = deque()

            def pop_fillers(k):
                for _ in range(k):
                    if filler:
                        filler.popleft()[1]()

            def drain_through(key):
                """Pop until no unit with the given key remains (they form a
                contiguous run in FIFO order)."""
                while any(u[0] == key for u in filler):
                    filler.popleft()[1]()

            # state deferred from the previous chunk
            pending = None  # (i, b, n, otp, pts, emit_av)

            def finish_pending():
                """Tail AVs for the previous chunk (emitted directly);
                normalization + phase-4 are returned as filler units."""
                nonlocal pending
                if pending is None:
                    return []
                pi, pb, pn, potp, ppts, pemit_av = pending
                pending = None
                for j in range(max(pn - AV_DELAY, 0), pn):
                    pemit_av(j)
                state = {}

                def norm_unit():
                    # both heads' Z rows sit at partition 64 of the two
                    # adjacent banks of potp: one f32r copy, PE broadcast
                    # (bch), then reciprocal + multiply per head on DVE.
                    rz = sbpool.tile([1, 2 * TC], FPR, tag="rz", bufs=2,
                                     name=f"rz{pi}")
                    with nc.allow_low_precision(reason="sums f32r"):
                        nc.scalar.copy(rz[:], potp[64:65, :])
                    ot = sbpool.tile([128, TC], BF, tag="ot", bufs=2,
                                     name=f"ot{pi}")
                    for h in range(2):
                        bchh = pspool.tile([64, TC], FP, tag="m", bufs=2,
                                           name=f"bch{h}_{pi}")
                        nc.tensor.matmul(bchh[:], onesr_sb[0:1, :],
                                         rz[0:1, h * TC:(h + 1) * TC],
                                         start=True, stop=True)
                        bcsh = sbpool.tile([64, TC], FP, tag=f"bcs{h}",
                                           bufs=2, name=f"bcs{h}_{pi}")
                        nc.vector.reciprocal_approx_fast(out=bcsh[:],
                                                         in_=bchh[:])
                        nc.vector.tensor_mul(
                            ot[h * 64:(h + 1) * 64, :],
                            potp[0:64, h * TC:(h + 1) * TC], bcsh[:])
                    state['ot'] = ot
                    state['ys'] = sbpool.tile([128, BT], BF, tag="ys",
                                              bufs=2, name=f"ys{pi}")

                def m_unit(m, last):
                    # PSUM evacuation engine: mostly DVE; ScalarE takes a
                    # couple per chunk (it has slack between exps), and on
                    # the final chunk (no exp left) strict alternation
                    # halves the copy-bound tail chain.
                    if last:
                        ceng = (nc.vector.tensor_copy,
                                nc.scalar.copy)[m % 2]
                    else:
                        ceng = (nc.vector.tensor_copy if m % 4 != 3
                                else nc.scalar.copy)

                    def emit():
                        yp = pspool.tile([128, TC], FP, tag="m", bufs=2,
                                         name=f"yp{pi}_{m}")
                        nc.tensor.matmul(
                            yp[:], wout_sb[:, m * 128:(m + 1) * 128],
                            state['ot'][:], start=True, stop=True)
                        ys = state['ys']
                        ceng(ys[:, m * TC:(m + 1) * TC], yp[:])
                        r0 = (m * 8 + pi) * 128
                        (nc.sync, nc.gpsimd)[m % 2].dma_start(
                            out=yt[r0:r0 + 128, :],
                            in_=ys[:, m * TC:(m + 1) * TC])
                    return emit

                last = pi == CHUNK_ORDER[-1]
                return [((pi, 'p4'), norm_unit)] + \
                    [((pi, 'p4'), m_unit(m, last)) for m in range(8)]

            for pos, tcx in enumerate(CHUNK_ORDER):
                b, qc = divmod(tcx, 4)
                t0 = tcx * TC
                n = 4 * (qc + 1)

                if pos == 0:
                    for _, u in make_phase1_units(tcx):
                        u()
                    for nxt in P1_AT_POS[pos]:
                        filler.extend(make_phase1_units(nxt))
                else:
                    # phase-1 of every chunk this one's scores depend on
                    # was queued as filler in earlier positions; drain
                    # any remainder of this chunk's own Q/K granules.
                    drain_through((tcx, 'qk'))
                    p1u = []
                    for nxt in P1_AT_POS[pos]:
                        p1u.extend(make_phase1_units(nxt))
                    p4u = finish_pending()  # [norm, m0..m7]
                    # interleave: the norm/out-proj units wait on a
                    # scalar->PE->DVE chain; dependency-free phase-1
                    # granules between them keep the PE FIFO head from
                    # blocking at chunk boundaries (HAM re-throttles).
                    merged = list(p1u[0:2])
                    i1 = 2
                    for u4 in p4u:
                        merged.append(u4)
                        merged.extend(p1u[i1:i1 + 2])
                        i1 += 2
                    merged.extend(p1u[i1:])
                    filler.extend(merged)
                    # emit through norm before the k-loop so its Z-row
                    # copy lands on the ScalarE queue ahead of this
                    # chunk's first exp.
                    pop_fillers(3 if p1u else 1)

                # both heads' AV accumulators in one 2-bank tile:
                # head h at columns [h*TC, (h+1)*TC); Z row at partition 64
                otp = pspool.tile([65, 2 * TC], FP, tag="av", bufs=1,
                                  name=f"otp{tcx}")
                pts = {}

                def emit_av(j, b=b, n=n, otp=otp, pts=pts):
                    kgj = b * 16 + j
                    pt, w = pts[j]
                    for h in range(2):
                        nc.tensor.matmul(
                            otp[:, h * TC + TC - w:h * TC + TC],
                            vaug[:, kgj * 130 + h * 65:
                                 kgj * 130 + h * 65 + 65],
                            pt[:, h * w:(h + 1) * w],
                            start=(j == 0), stop=(j == n - 1),
                            skip_group_check=True,
                        )

                for kt in range(n):
                    kg = b * 16 + kt
                    # diagonal k-tile v only attends to q >= v*128: shrink
                    # the free width of scores/exp/AV accordingly
                    v = kt - (n - 4)
                    w = TC - 128 * v if v > 0 else TC
                    s = pspool.tile([128, 2 * TC], FP, tag="s", bufs=2,
                                    name=f"s{tcx}_{kt}")
                    for h in range(2):
                        # head h stays at fixed offset h*TC: a matmul
                        # destination must not cross a PSUM bank boundary
                        nc.tensor.matmul(
                            s[:, h * TC:h * TC + w],
                            KT[h * 64:(h + 1) * 64, kg * 128:(kg + 1) * 128],
                            QT[h * 64:(h + 1) * 64, t0 + TC - w:t0 + TC],
                            start=True, stop=True,
                        )
                    # V granules of this chunk must land before the
                    # diagonal AV matmuls need vaug
                    if kt >= max(n - 6, 0):
                        drain_through((tcx, 'v'))
                    # adaptive filler drain across remaining k-slots
                    slots = n - kt
                    pop_fillers(-(-len(filler) // slots) if filler else 0)
                    pt = sbpool.tile([128, 2 * TC], BF, tag="pt", bufs=8,
                                     name=f"pt{tcx}_{kt}")
                    nc.scalar.activation(
                        pt[:, 0:2 * w].rearrange("p (h q) -> p h q", h=2),
                        s[:].rearrange("p (h q) -> p h q", h=2)[:, :, 0:w],
                        ACT.Exp, scale=0.125)
                    if kt >= n - 4:
                        # multiplicative causal mask on the first 128 q
                        # columns of each head's block (the in-tile triangle)
                        nc.vector.tensor_mul(
                            pt[:, 0:2 * w].rearrange(
                                "p (h q) -> p h q", h=2)[:, :, 0:128],
                            pt[:, 0:2 * w].rearrange(
                                "p (h q) -> p h q", h=2)[:, :, 0:128],
                            masks_sb[:].rearrange(
                                "p (h q) -> p h q", h=2),
                        )
                    pts[kt] = (pt, w)
                    if kt >= AV_DELAY:
                        emit_av(kt - AV_DELAY)
                pending = (tcx, b, n, otp, pts, emit_av)

            pop_fillers(len(filler))
            for _, u in finish_pending():
                u()
    nc.compile()
    return nc


def make_in_maps(x, w_qkv, b_qkv, w_out):
    x = np.ascontiguousarray(np.asarray(x, np.float32).reshape(BT, C))
    xT = np.ascontiguousarray(x.T).astype(ml_dtypes.bfloat16)
    w_qkv = np.asarray(w_qkv, np.float32)
    b_qkv = np.asarray(b_qkv, np.float32)
    w_out = np.asarray(w_out, np.float32)

    # In-tile causal triangle, duplicated per head: masks[p, h*128 + q] =
    # 1 if p <= q else 0 (multiplies the exp output: exact causal zeroing)
    kk = np.arange(128)[:, None, None]
    qq = np.arange(128)[None, None, :]
    mask = np.where(kk <= qq, 1.0, 0.0) * np.ones((1, 2, 1))
    mask = np.ascontiguousarray(
        mask.reshape(128, 256)).astype(ml_dtypes.bfloat16)

    # wc[p, k*384 + g*128 + j] = w_qkv[k*128 + p, g*1024 + c0 + j]
    w4 = w_qkv.reshape(8, 128, 3, 1024)

    in_maps = []
    for c in range(NCORES):
        c0 = c * 128
        wcs = np.ascontiguousarray(
            w4[:, :, :, c0:c0 + 128].transpose(1, 0, 2, 3).reshape(128, 3072)
        ).astype(ml_dtypes.bfloat16)
        bq = np.ascontiguousarray(
            b_qkv.reshape(3, 1024)[:, c0:c0 + 128].T)
        bv = np.ascontiguousarray(
            b_qkv[2 * 1024 + c0:2 * 1024 + c0 + 128].reshape(1, 128)
        ).astype(ml_dtypes.bfloat16)
        in_maps.append({
            "xt": xT,
            "wc": wcs,
            "wout": np.ascontiguousarray(
                w_out[c0:c0 + 128, :]).astype(ml_dtypes.bfloat16),
            "bqkv": bq,
            "bvrow": bv,
            "masks": mask,
            "onesr": np.ones((1, 64), np.float32),
            "onesb": np.ones((1, 128), ml_dtypes.bfloat16),
        })
    return in_maps


_NC_CACHE = None


def kernel(x, w_qkv, b_qkv, w_out, b_out):
    global _NC_CACHE, LAST_RESULTS
    if _NC_CACHE is None:
        _NC_CACHE = build_nc()
    nc = _NC_CACHE

    in_maps = make_in_maps(x, w_qkv, b_qkv, w_out)

    res = run_bass_kernel_spmd(
        nc, in_maps, list(range(NCORES)),
        trace=bool(os.environ.get("BASS_TRACE")),
    )
    LAST_RESULTS = res

    acc = np.zeros((C, BT), np.float64)
    for out_map in res.results:
        # yt[(m*8 + i)*128 + p, t] -> y_partial[m*128 + p, i*512 + t]
        yp = out_map["yt"].astype(np.float32)
        yp = yp.reshape(8, 8, 128, 512).transpose(0, 2, 1, 3).reshape(C, BT)
        acc += yp
    y = acc.T.astype(np.float32) + np.asarray(b_out, np.float32)[None, :]
    return y.reshape(B, T, C)
